# revision 11
# baseline (speedup 1.0000x reference)
"""Trainium2 kernel for CondensedLinearFineGrainedSparseOp:
    out[b,s,o] = sum_k x[b,s,k] * weight[o,k] + bias[o]
with x [8, 2048, 4096] f32, weight [4096, 4096] f32 (90% zeros, stored
dense), bias [4096] f32 -> out [8, 2048, 4096] f32.

Strategy: data-parallel shard over tokens (B*S = 16384 -> 2048 per core)
across 8 NeuronCores; weight/bias replicated. The unstructured 10%
sparsity is not exploitable on the 128x128 PE array (any >=8x8 block of
the mask is nonempty with overwhelming probability), so each core runs a
dense [2048 x 4096 x 4096] GEMM in bf16 with fp32 PSUM accumulation.

Per-core kernel: out[t,o] with t on PSUM partitions. Stationary operand =
x^T tile [128k, 128t]; moving operand = W^T tile [128k, 512o]. o is
processed in 5 phases (512,512,1024,1024,1024); each phase's W k-tiles
are SBUF resident (double-buffered across phases) while x^T streams per
t-tile (re-read once per phase).

Cold-start engineering (the PE roofline for this GEMM is ~874us, so the
only recoverable time is at the edges):
 - W arrives in FEW LARGE DMAs (128KB singles for the first 4 k-tiles,
   then 0.5-2MB groups). The Tile runtime only keeps 8 HWDGE transfers
   in flight, so many small DMAs cap the cold W stream at ~50GB/s;
   grouped pieces keep 8-16MB in flight instead of 1MB.
 - One phase-0 W group and the bias ride the gpsimd SWDGE queue so the
   two HWDGE rings (sync/scalar) only carry the critical W stream.
 - Bias is split 1KB + 3KB per partition so the first eviction never
   waits on the full 16KB/partition bias transfer.
 - ~10 dummy 128-col matmuls on a memset scratch tile warm the PE
   p-state during the 6-11us window where DMA queues initialize, so the
   first real matmuls run at full clock instead of ~1.2GHz.
 - The last tile's eviction + output DMA is split in half across both
   HWDGE queues to shrink the drain tail.
"""

import numpy as np
import ml_dtypes

import concourse.mybir as mybir
import concourse.tile as tile
from concourse import bacc
from concourse.bass import ts
from concourse.bass_utils import run_bass_kernel_spmd

P = 128
NCORES = 8
B, S, DIN, DOUT = 8, 2048, 4096, 4096
T = B * S // NCORES          # tokens per core
KT = DIN // P                # 32 contraction tiles
NT = T // P                  # 16 token tiles per core

BF16 = mybir.dt.bfloat16
F32 = mybir.dt.float32

# Phases over the output-feature dim. First two are 512 wide so the
# cold-start W mass is small; steady state is 1024 (W double-buffered).
PHASES = [(0, 512), (512, 512), (1024, 1024), (2048, 1024), (3072, 1024)]

# W is host-packed into contiguous per-partition "pieces", each one DMA.
# Piece = (phase_idx, k_start, n_ktiles). Order here == DMA issue order
# == layout order in the packed dram tensor.
def _make_pieces():
    pieces = []
    # phase 0 (olen 512): need-ordered, just-in-time sizes. Singles
    # (128KB) unblock the PE k-by-k through the cold DMA ramp; the
    # back half rides 512KB groups; the tail goes on the SWDGE queue.
    for k in range(6):
        pieces.append((0, k, 1))
    for k in range(6, 26, 4):
        pieces.append((0, k, 4))
    pieces.append((0, 26, 6))
    # phase 1 (olen 512): groups of 8
    for k in range(0, KT, 8):
        pieces.append((1, k, 8))
    # phases 2-4 (olen 1024): groups of 4
    for ph in (2, 3, 4):
        for k in range(0, KT, 4):
            pieces.append((ph, k, 4))
    return pieces

PIECES = _make_pieces()
PIECE_OFF = []
_off = 0
for _ph, _k, _g in PIECES:
    PIECE_OFF.append(_off)
    _off += _g * PHASES[_ph][1]
WPACK_F = _off               # free elems per partition (== DIN*DOUT/128)

_NC = None
LAST_RESULT = None


def _build_nc():
    nc = bacc.Bacc("TRN2", target_bir_lowering=False, debug=False)
    # x pre-tiled on host: xt[t, p, ks, i] = x[t*128+i, ks*128+p]
    # -> each t-tile DMA is one fully linear 1MB read
    xt = nc.dram_tensor("xt", [NT, P, KT, P], BF16, kind="ExternalInput")
    wp = nc.dram_tensor("wpack", [P, WPACK_F], BF16, kind="ExternalInput")
    bias = nc.dram_tensor("bias_rep", [P, DOUT], F32, kind="ExternalInput")
    out = nc.dram_tensor("out", [T, DOUT], F32, kind="ExternalOutput")

    with tile.TileContext(nc) as tc:
        with (
            tc.tile_pool(name="wpool", bufs=16) as wpool,
            tc.tile_pool(name="xpool", bufs=4) as xpool,
            tc.tile_pool(name="bpool", bufs=1) as bpool,
            tc.tile_pool(name="opool", bufs=3) as opool,
            tc.tile_pool(name="psum", bufs=8, space="PSUM") as psum_pool,
        ):
            # Tiny warmup DMA on each queue: absorbs cold DGE/queue init
            # on throwaway transfers instead of the critical first W/x.
            for i, eng in enumerate((nc.sync, nc.scalar, nc.gpsimd)):
                wu = bpool.tile([P, 8], F32, tag=f"wu{i}", name=f"wu{i}")
                eng.dma_start(wu[:], bias.ap()[:, ts(i, 8)])

            # PE p-state warmup: ~10 dummy matmuls on a zeroed scratch
            # tile keep the PE active while the real W/x DMAs are in
            # flight, so the first real matmuls run at full clock.
            scr = bpool.tile([P, P], BF16, tag="scr", name="scr")
            nc.gpsimd.memset(scr[:], 0)
            dummy_acc = psum_pool.tile([P, 512], F32, tag="acc", name="acc")
            for _ in range(10):
                nc.tensor.matmul(
                    dummy_acc[:, :P], scr[:], scr[:], start=True, stop=True
                )

            # First x tile in 4 chunks ahead of everything else on the
            # SWDGE queue so the PE's first stationary operand arrives
            # within ~10us even while the cold W stream saturates HBM.
            x_first = xpool.tile([P, KT, P], BF16, tag="x", name="x_first")
            for c in range(4):
                nc.gpsimd.dma_start(
                    x_first[:, ts(c, KT // 4), :],
                    xt.ap()[0, :, ts(c, KT // 4), :],
                )

            bias_sb = bpool.tile([P, DOUT], F32)

            # ---- W piece DMAs -> per-(phase,k) moving-operand slices
            w_slice = {}           # (ph, k) -> (tile, elem offset)
            piece_by_phase = [[] for _ in PHASES]
            for pi, (ph, k0, g) in enumerate(PIECES):
                piece_by_phase[ph].append((pi, k0, g))

            def issue_piece(ph, idx, eng):
                olen = PHASES[ph][1]
                pi, k0, g = piece_by_phase[ph][idx]
                wtile = wpool.tile(
                    [P, g * olen], BF16, tag="w", name="w",
                    padded_shape=[P, 4096],
                )
                eng.dma_start(
                    wtile[:],
                    wp.ap()[:, PIECE_OFF[pi] : PIECE_OFF[pi] + g * olen],
                )
                for j in range(g):
                    w_slice[(ph, k0 + j)] = (wtile, j * olen)

            def issue_phase_w(ph, engines, last_on_gpsimd=False):
                n = len(piece_by_phase[ph])
                for idx in range(n):
                    if last_on_gpsimd and idx == n - 1:
                        eng = nc.gpsimd
                    else:
                        eng = engines[idx % len(engines)]
                    issue_piece(ph, idx, eng)

            for ph, (o0, olen) in enumerate(PHASES):
                if ph == 0:
                    # Phase-0 W: fine-grained singles first so the PE
                    # unblocks k-by-k, the last group on the SWDGE queue
                    # so both HWDGE rings stream the middle groups.
                    # gpsimd order: scr/x_first (above), W k24-31,
                    # bias[0:1024], then the x t-tile stream below.
                    issue_phase_w(0, [nc.sync, nc.scalar],
                                  last_on_gpsimd=True)
                    # first bias half rides sync behind phase-0's W
                    # (landed well before the first eviction needs it);
                    # NOT on gpsimd — that queue must feed the x stream
                    nc.sync.dma_start(
                        bias_sb[:, 0:1024], bias.ap()[:, 0:1024]
                    )
                # Phases >= 1 are issued near the end of t==0 of the
                # previous phase (below): by then their pool slots are
                # provably free (no WAR wait can block the HWDGE FIFO)
                # and the stream still has nearly a full phase of
                # compute (~95-200us) to land before first use.

                banks = [512] * (olen // 512)
                for t in range(NT):
                    if ph == 0 and t == 0:
                        xtile = x_first
                    else:
                        xtile = xpool.tile([P, KT, P], BF16, tag="x")
                        nc.gpsimd.dma_start(xtile[:], xt.ap()[t])

                    accs = [
                        psum_pool.tile([P, 512], F32, tag="acc", name="acc")
                        for _ in banks
                    ]
                    for k in range(KT):
                        wtile, eoff = w_slice[(ph, k)]
                        for b, blen in enumerate(banks):
                            nc.tensor.matmul(
                                accs[b][:],
                                xtile[:, k, :],                      # stationary
                                wtile[:, eoff + b * blen : eoff + (b + 1) * blen],
                                start=(k == 0),
                                stop=(k == KT - 1),
                            )

                    last = ph == len(PHASES) - 1 and t == NT - 1
                    if last:
                        # split the final eviction across both HWDGE
                        # queues to shrink the drain tail
                        for b, blen in enumerate(banks):
                            osb = opool.tile(
                                [P, blen], F32, tag="o", name="o",
                                padded_shape=[P, 1024],
                            )
                            nc.vector.tensor_add(
                                osb[:],
                                accs[b][:],
                                bias_sb[:, o0 + b * blen : o0 + (b + 1) * blen],
                            )
                            eng = nc.sync if b % 2 == 0 else nc.scalar
                            eng.dma_start(
                                out.ap()[
                                    ts(t, P), o0 + b * blen : o0 + (b + 1) * blen
                                ],
                                osb[:],
                            )
                    else:
                        osb = opool.tile(
                            [P, olen], F32, tag="o", name="o",
                            padded_shape=[P, 1024],
                        )
                        for b, blen in enumerate(banks):
                            nc.vector.tensor_add(
                                osb[:, ts(b, blen)],
                                accs[b][:],
                                bias_sb[:, o0 + b * blen : o0 + (b + 1) * blen],
                            )
                        eng = nc.sync if t % 2 == 0 else nc.scalar
                        eng.dma_start(
                            out.ap()[ts(t, P), o0 : o0 + olen], osb[:]
                        )

                    if ph + 1 < len(PHASES) and t % 2 == 0:
                        # Prefetch the next phase's W, PACED at one
                        # piece per two t-tiles: a bulk prefetch burst
                        # monopolizes the shared SDMA engines and
                        # starves the x stream (measured 3-7us PE
                        # stalls per t-tile). Slots were freed a full
                        # phase ago, so no WAR wait can stall the
                        # HWDGE FIFO behind this.
                        idx = t // 2
                        if idx < len(piece_by_phase[ph + 1]):
                            eng = nc.sync if idx % 2 == 0 else nc.scalar
                            issue_piece(ph + 1, idx, eng)
                    if ph == 0 and t == 1:
                        # rest of the bias, first needed in phase 2
                        nc.scalar.dma_start(
                            bias_sb[:, 1024:DOUT],
                            bias.ap()[:, 1024:DOUT],
                        )

    nc.compile()
    return nc


def _pack_weight(weight):
    wt = weight.T.astype(ml_dtypes.bfloat16)           # [DIN, DOUT]
    wk = wt.reshape(KT, P, DOUT)                       # [k, p, o]
    wpack = np.empty((P, WPACK_F), dtype=ml_dtypes.bfloat16)
    for pi, (ph, k0, g) in enumerate(PIECES):
        o0, olen = PHASES[ph]
        blk = wk[k0 : k0 + g, :, o0 : o0 + olen]       # [g, p, olen]
        wpack[:, PIECE_OFF[pi] : PIECE_OFF[pi] + g * olen] = (
            blk.transpose(1, 0, 2).reshape(P, g * olen)
        )
    return np.ascontiguousarray(wpack)


def kernel(x, weight, bias):
    global _NC, LAST_RESULT
    if _NC is None:
        _NC = _build_nc()

    X = np.ascontiguousarray(x.reshape(B * S, DIN))
    wpack = _pack_weight(weight)
    bias_rep = np.ascontiguousarray(
        np.broadcast_to(bias.astype(np.float32), (P, DOUT))
    )
    in_maps = []
    for c in range(NCORES):
        xc = X[c * T : (c + 1) * T].astype(ml_dtypes.bfloat16)
        # [t-tile, p(=k%128), ks, i(=token%128)]
        xt_c = np.ascontiguousarray(
            xc.reshape(NT, P, KT, P).transpose(0, 3, 2, 1)
        )
        in_maps.append({"xt": xt_c, "wpack": wpack, "bias_rep": bias_rep})

    last_err = None
    for _attempt in range(2):
        try:
            res = run_bass_kernel_spmd(_NC, in_maps, list(range(NCORES)))
            break
        except Exception as e:  # transient NRT device errors: retry once
            last_err = e
    else:
        raise last_err
    LAST_RESULT = res

    out = np.concatenate([res.results[c]["out"] for c in range(NCORES)], axis=0)
    return out.reshape(B, S, DOUT).astype(np.float32, copy=False)


# revision 15
# speedup vs baseline: 1.0010x; 1.0010x over previous
"""Trainium2 kernel for CondensedLinearFineGrainedSparseOp:
    out[b,s,o] = sum_k x[b,s,k] * weight[o,k] + bias[o]
with x [8, 2048, 4096] f32, weight [4096, 4096] f32 (90% zeros, stored
dense), bias [4096] f32 -> out [8, 2048, 4096] f32.

Strategy: data-parallel shard over tokens (B*S = 16384 -> 2048 per core)
across 8 NeuronCores; weight/bias replicated. The unstructured 10%
sparsity is not exploitable on the 128x128 PE array (any >=8x8 block of
the mask is nonempty with overwhelming probability), so each core runs a
dense [2048 x 4096 x 4096] GEMM in bf16 with fp32 PSUM accumulation.

Per-core kernel: out[t,o] with t on PSUM partitions. Stationary operand =
x^T tile [128k, 128t]; moving operand = W^T tile [128k, 512o]. o is
processed in 5 phases (512,512,1024,1024,1024); each phase's W k-tiles
are SBUF resident (double-buffered across phases) while x^T streams per
t-tile (re-read once per phase).

Cold-start engineering (the PE roofline for this GEMM is ~874us, so the
only recoverable time is at the edges):
 - W arrives in FEW LARGE DMAs (128KB singles for the first 4 k-tiles,
   then 0.5-2MB groups). The Tile runtime only keeps 8 HWDGE transfers
   in flight, so many small DMAs cap the cold W stream at ~50GB/s;
   grouped pieces keep 8-16MB in flight instead of 1MB.
 - One phase-0 W group and the bias ride the gpsimd SWDGE queue so the
   two HWDGE rings (sync/scalar) only carry the critical W stream.
 - Bias is split 1KB + 3KB per partition so the first eviction never
   waits on the full 16KB/partition bias transfer.
 - ~10 dummy 128-col matmuls on a memset scratch tile warm the PE
   p-state during the 6-11us window where DMA queues initialize, so the
   first real matmuls run at full clock instead of ~1.2GHz.
 - The last tile's eviction + output DMA is split in half across both
   HWDGE queues to shrink the drain tail.
"""

import numpy as np
import ml_dtypes

import concourse.mybir as mybir
import concourse.tile as tile
from concourse import bacc
from concourse.bass import ts
from concourse.bass_utils import run_bass_kernel_spmd

P = 128
NCORES = 8
B, S, DIN, DOUT = 8, 2048, 4096, 4096
T = B * S // NCORES          # tokens per core
KT = DIN // P                # 32 contraction tiles
NT = T // P                  # 16 token tiles per core

BF16 = mybir.dt.bfloat16
F32 = mybir.dt.float32

# Phases over the output-feature dim. First two are 512 wide so the
# cold-start W mass is small; steady state is 1024 (W double-buffered).
PHASES = [(0, 512), (512, 512), (1024, 1024), (2048, 1024), (3072, 1024)]

# W is host-packed into contiguous per-partition "pieces", each one DMA.
# Piece = (phase_idx, k_start, n_ktiles). Order here == DMA issue order
# == layout order in the packed dram tensor.
def _make_pieces():
    pieces = []
    # phase 0 (olen 512): need-ordered, just-in-time sizes. Singles
    # (128KB) unblock the PE k-by-k through the cold DMA ramp; the
    # back half rides 512KB groups; the tail goes on the SWDGE queue.
    for k in range(6):
        pieces.append((0, k, 1))
    for k in range(6, 26, 4):
        pieces.append((0, k, 4))
    pieces.append((0, 26, 6))
    # phase 1 (olen 512): groups of 8
    for k in range(0, KT, 8):
        pieces.append((1, k, 8))
    # phases 2-4 (olen 1024): groups of 4
    for ph in (2, 3, 4):
        for k in range(0, KT, 4):
            pieces.append((ph, k, 4))
    return pieces

PIECES = _make_pieces()
PIECE_OFF = []
_off = 0
for _ph, _k, _g in PIECES:
    PIECE_OFF.append(_off)
    _off += _g * PHASES[_ph][1]
WPACK_F = _off               # free elems per partition (== DIN*DOUT/128)

_NC = None
LAST_RESULT = None


def _build_nc():
    nc = bacc.Bacc("TRN2", target_bir_lowering=False, debug=False)
    # x pre-tiled on host: xt[t, p, ks, i] = x[t*128+i, ks*128+p]
    # -> each t-tile DMA is one fully linear 1MB read
    xt = nc.dram_tensor("xt", [NT, P, KT, P], BF16, kind="ExternalInput")
    wp = nc.dram_tensor("wpack", [P, WPACK_F], BF16, kind="ExternalInput")
    bias = nc.dram_tensor("bias_rep", [P, DOUT], F32, kind="ExternalInput")
    out = nc.dram_tensor("out", [T, DOUT], F32, kind="ExternalOutput")

    with tile.TileContext(nc) as tc:
        with (
            tc.tile_pool(name="wpool", bufs=16) as wpool,
            tc.tile_pool(name="xpool", bufs=4) as xpool,
            tc.tile_pool(name="bpool", bufs=1) as bpool,
            tc.tile_pool(name="opool", bufs=3) as opool,
            tc.tile_pool(name="psum", bufs=8, space="PSUM") as psum_pool,
        ):
            # Tiny warmup DMA on each queue: absorbs cold DGE/queue init
            # on throwaway transfers instead of the critical first W/x.
            for i, eng in enumerate((nc.sync, nc.scalar, nc.gpsimd)):
                wu = bpool.tile([P, 8], F32, tag=f"wu{i}", name=f"wu{i}")
                eng.dma_start(wu[:], bias.ap()[:, ts(i, 8)])

            # PE p-state warmup: ~10 dummy matmuls on a zeroed scratch
            # tile keep the PE active while the real W/x DMAs are in
            # flight, so the first real matmuls run at full clock.
            scr = bpool.tile([P, P], BF16, tag="scr", name="scr")
            nc.gpsimd.memset(scr[:], 0)
            dummy_acc = psum_pool.tile([P, 512], F32, tag="acc", name="acc")
            for _ in range(10):
                nc.tensor.matmul(
                    dummy_acc[:, :P], scr[:], scr[:], start=True, stop=True
                )

            # First x tile in 4 chunks ahead of everything else on the
            # sync HWDGE queue (~0.6us first-byte vs ~2us SWDGE, and
            # measured 2x faster through the cold DMA-engine ramp) so
            # the PE's first stationary operand arrives within ~10us.
            x_first = xpool.tile([P, KT, P], BF16, tag="x", name="x_first")
            for c in range(4):
                nc.sync.dma_start(
                    x_first[:, ts(c, KT // 4), :],
                    xt.ap()[0, :, ts(c, KT // 4), :],
                )

            bias_sb = bpool.tile([P, DOUT], F32)

            # ---- W piece DMAs -> per-(phase,k) moving-operand slices
            w_slice = {}           # (ph, k) -> (tile, elem offset)
            piece_by_phase = [[] for _ in PHASES]
            for pi, (ph, k0, g) in enumerate(PIECES):
                piece_by_phase[ph].append((pi, k0, g))

            def issue_piece(ph, idx, eng):
                olen = PHASES[ph][1]
                pi, k0, g = piece_by_phase[ph][idx]
                wtile = wpool.tile(
                    [P, g * olen], BF16, tag="w", name="w",
                    padded_shape=[P, 4096],
                )
                eng.dma_start(
                    wtile[:],
                    wp.ap()[:, PIECE_OFF[pi] : PIECE_OFF[pi] + g * olen],
                )
                for j in range(g):
                    w_slice[(ph, k0 + j)] = (wtile, j * olen)

            def issue_phase_w(ph, engines, last_on_gpsimd=False):
                n = len(piece_by_phase[ph])
                for idx in range(n):
                    if last_on_gpsimd and idx == n - 1:
                        eng = nc.gpsimd
                    else:
                        eng = engines[idx % len(engines)]
                    issue_piece(ph, idx, eng)

            for ph, (o0, olen) in enumerate(PHASES):
                if ph == 0:
                    # Phase-0 W, need-ordered: singles k0-5 on scalar
                    # (sync's first lanes carry the x chunks), groups
                    # interleaved sync/scalar, tail group on SWDGE.
                    ph0_eng = [nc.scalar] * 6 + [
                        nc.sync, nc.scalar, nc.sync, nc.scalar,
                        nc.sync, nc.gpsimd,
                    ]
                    for idx, eng in enumerate(ph0_eng):
                        issue_piece(0, idx, eng)
                    # first bias half behind phase-0's W on scalar
                    # (lands well before the first eviction needs it)
                    nc.scalar.dma_start(
                        bias_sb[:, 0:1024], bias.ap()[:, 0:1024]
                    )
                # Phases >= 1 are issued near the end of t==0 of the
                # previous phase (below): by then their pool slots are
                # provably free (no WAR wait can block the HWDGE FIFO)
                # and the stream still has nearly a full phase of
                # compute (~95-200us) to land before first use.

                banks = [512] * (olen // 512)
                for t in range(NT):
                    if ph == 0 and t == 0:
                        xtile = x_first
                    else:
                        # alternate the x stream across SWDGE and the
                        # sync HWDGE ring so W prefetch bursts on one
                        # queue cannot starve the stationary operand
                        xtile = xpool.tile([P, KT, P], BF16, tag="x")
                        xeng = nc.sync if t % 2 == 1 else nc.gpsimd
                        xeng.dma_start(xtile[:], xt.ap()[t])

                    accs = [
                        psum_pool.tile([P, 512], F32, tag="acc", name="acc")
                        for _ in banks
                    ]
                    for k in range(KT):
                        wtile, eoff = w_slice[(ph, k)]
                        for b, blen in enumerate(banks):
                            nc.tensor.matmul(
                                accs[b][:],
                                xtile[:, k, :],                      # stationary
                                wtile[:, eoff + b * blen : eoff + (b + 1) * blen],
                                start=(k == 0),
                                stop=(k == KT - 1),
                            )

                    last = ph == len(PHASES) - 1 and t == NT - 1
                    if last:
                        # split the final eviction across both HWDGE
                        # queues to shrink the drain tail
                        for b, blen in enumerate(banks):
                            osb = opool.tile(
                                [P, blen], F32, tag="o", name="o",
                                padded_shape=[P, 1024],
                            )
                            nc.vector.tensor_add(
                                osb[:],
                                accs[b][:],
                                bias_sb[:, o0 + b * blen : o0 + (b + 1) * blen],
                            )
                            eng = nc.sync if b % 2 == 0 else nc.scalar
                            eng.dma_start(
                                out.ap()[
                                    ts(t, P), o0 + b * blen : o0 + (b + 1) * blen
                                ],
                                osb[:],
                            )
                    else:
                        osb = opool.tile(
                            [P, olen], F32, tag="o", name="o",
                            padded_shape=[P, 1024],
                        )
                        for b, blen in enumerate(banks):
                            nc.vector.tensor_add(
                                osb[:, ts(b, blen)],
                                accs[b][:],
                                bias_sb[:, o0 + b * blen : o0 + (b + 1) * blen],
                            )
                        # out rides scalar only: eviction-paced, so it
                        # also throttles the W prefetch pieces queued
                        # behind it (sync's FIFO stays clear for x)
                        nc.scalar.dma_start(
                            out.ap()[ts(t, P), o0 : o0 + olen], osb[:]
                        )

                    if ph + 1 < len(PHASES) and t % 2 == 0:
                        # Prefetch the next phase's W, PACED at one
                        # piece per two t-tiles: a bulk prefetch burst
                        # monopolizes the shared SDMA engines and
                        # starves the x stream (measured 3-7us PE
                        # stalls per t-tile). Slots were freed a full
                        # phase ago, so no WAR wait can stall the
                        # HWDGE FIFO behind this.
                        idx = t // 2
                        if idx < len(piece_by_phase[ph + 1]):
                            eng = nc.sync if idx % 2 == 0 else nc.scalar
                            issue_piece(ph + 1, idx, eng)
                    if ph == 0 and t == 1:
                        # rest of the bias, first needed in phase 2
                        nc.scalar.dma_start(
                            bias_sb[:, 1024:DOUT],
                            bias.ap()[:, 1024:DOUT],
                        )

    nc.compile()
    return nc


def _pack_weight(weight):
    wt = weight.T.astype(ml_dtypes.bfloat16)           # [DIN, DOUT]
    wk = wt.reshape(KT, P, DOUT)                       # [k, p, o]
    wpack = np.empty((P, WPACK_F), dtype=ml_dtypes.bfloat16)
    for pi, (ph, k0, g) in enumerate(PIECES):
        o0, olen = PHASES[ph]
        blk = wk[k0 : k0 + g, :, o0 : o0 + olen]       # [g, p, olen]
        wpack[:, PIECE_OFF[pi] : PIECE_OFF[pi] + g * olen] = (
            blk.transpose(1, 0, 2).reshape(P, g * olen)
        )
    return np.ascontiguousarray(wpack)


def kernel(x, weight, bias):
    global _NC, LAST_RESULT
    if _NC is None:
        _NC = _build_nc()

    X = np.ascontiguousarray(x.reshape(B * S, DIN))
    wpack = _pack_weight(weight)
    bias_rep = np.ascontiguousarray(
        np.broadcast_to(bias.astype(np.float32), (P, DOUT))
    )
    in_maps = []
    for c in range(NCORES):
        xc = X[c * T : (c + 1) * T].astype(ml_dtypes.bfloat16)
        # [t-tile, p(=k%128), ks, i(=token%128)]
        xt_c = np.ascontiguousarray(
            xc.reshape(NT, P, KT, P).transpose(0, 3, 2, 1)
        )
        in_maps.append({"xt": xt_c, "wpack": wpack, "bias_rep": bias_rep})

    last_err = None
    for _attempt in range(2):
        try:
            res = run_bass_kernel_spmd(_NC, in_maps, list(range(NCORES)))
            break
        except Exception as e:  # transient NRT device errors: retry once
            last_err = e
    else:
        raise last_err
    LAST_RESULT = res

    out = np.concatenate([res.results[c]["out"] for c in range(NCORES)], axis=0)
    return out.reshape(B, S, DOUT).astype(np.float32, copy=False)


# revision 20
# speedup vs baseline: 1.0239x; 1.0229x over previous
"""Trainium2 kernel for CondensedLinearFineGrainedSparseOp:
    out[b,s,o] = sum_k x[b,s,k] * weight[o,k] + bias[o]
with x [8, 2048, 4096] f32, weight [4096, 4096] f32 (90% zeros, stored
dense), bias [4096] f32 -> out [8, 2048, 4096] f32.

Strategy: data-parallel shard over tokens (B*S = 16384 -> 2048 per core)
across 8 NeuronCores; weight/bias replicated. The unstructured 10%
sparsity is not exploitable on the 128x128 PE array (any >=8x8 block of
the mask is nonempty with overwhelming probability), so each core runs a
dense [2048 x 4096 x 4096] GEMM in bf16 with fp32 PSUM accumulation.

Per-core kernel: out[t,o] with t on PSUM partitions. Stationary operand =
x^T tile [128k, 128t]; moving operand = W^T tile [128k, 512o]. o is
processed in 5 phases (512,512,1024,1024,1024); each phase's W k-tiles
are SBUF resident (double-buffered across phases) while x^T streams per
t-tile (re-read once per phase).

Cold-start engineering (the PE roofline for this GEMM is ~874us, so the
only recoverable time is at the edges):
 - W arrives in FEW LARGE DMAs (128KB singles for the first 4 k-tiles,
   then 0.5-2MB groups). The Tile runtime only keeps 8 HWDGE transfers
   in flight, so many small DMAs cap the cold W stream at ~50GB/s;
   grouped pieces keep 8-16MB in flight instead of 1MB.
 - One phase-0 W group and the bias ride the gpsimd SWDGE queue so the
   two HWDGE rings (sync/scalar) only carry the critical W stream.
 - Bias is split 1KB + 3KB per partition so the first eviction never
   waits on the full 16KB/partition bias transfer.
 - ~10 dummy 128-col matmuls on a memset scratch tile warm the PE
   p-state during the 6-11us window where DMA queues initialize, so the
   first real matmuls run at full clock instead of ~1.2GHz.
 - The last tile's eviction + output DMA is split in half across both
   HWDGE queues to shrink the drain tail.
"""

import numpy as np
import ml_dtypes

import concourse.mybir as mybir
import concourse.tile as tile
from concourse import bacc
from concourse.bass import ts
from concourse.bass_utils import run_bass_kernel_spmd

P = 128
NCORES = 8
B, S, DIN, DOUT = 8, 2048, 4096, 4096
T = B * S // NCORES          # tokens per core
KT = DIN // P                # 32 contraction tiles
NT = T // P                  # 16 token tiles per core

BF16 = mybir.dt.bfloat16
F32 = mybir.dt.float32

# Phases over the output-feature dim. First two are 512 wide so the
# cold-start W mass is small; steady state is 1024 (W double-buffered).
PHASES = [(0, 512), (512, 512), (1024, 1024), (2048, 1024), (3072, 1024)]

# W is host-packed into contiguous per-partition "pieces", each one DMA.
# Piece = (phase_idx, k_start, n_ktiles). Order here == DMA issue order
# == layout order in the packed dram tensor.
def _make_pieces():
    pieces = []
    # phase 0 (olen 512): need-ordered, just-in-time sizes. Singles
    # (128KB) unblock the PE k-by-k through the cold DMA ramp; the
    # back half rides 512KB groups; the tail goes on the SWDGE queue.
    for k in range(6):
        pieces.append((0, k, 1))
    for k in range(6, 30, 4):
        pieces.append((0, k, 4))
    pieces.append((0, 30, 2))
    # phases 1-4: 16 pieces each of ~512KB — big enough that the
    # 8-semaphore-lane in-flight cap sustains full HBM rate, small
    # enough that a piece never monopolizes lanes/engines against the
    # x stream (measured 3-11us PE stalls with 1-2MB pieces)
    for k in range(0, KT, 2):
        pieces.append((1, k, 2))
    for ph in (2, 3, 4):
        for k in range(0, KT, 2):
            pieces.append((ph, k, 2))
    return pieces

PIECES = _make_pieces()
PIECE_OFF = []
_off = 0
for _ph, _k, _g in PIECES:
    PIECE_OFF.append(_off)
    _off += _g * PHASES[_ph][1]
WPACK_F = _off               # free elems per partition (== DIN*DOUT/128)

_NC = None
LAST_RESULT = None


def _build_nc():
    nc = bacc.Bacc("TRN2", target_bir_lowering=False, debug=False)
    # x pre-tiled on host: xt[t, p, ks, i] = x[t*128+i, ks*128+p]
    # -> each t-tile DMA is one fully linear 1MB read
    xt = nc.dram_tensor("xt", [NT, P, KT, P], BF16, kind="ExternalInput")
    wp = nc.dram_tensor("wpack", [P, WPACK_F], BF16, kind="ExternalInput")
    bias = nc.dram_tensor("bias_rep", [P, DOUT], F32, kind="ExternalInput")
    out = nc.dram_tensor("out", [T, DOUT], F32, kind="ExternalOutput")

    with tile.TileContext(nc) as tc:
        with (
            tc.tile_pool(name="wpool", bufs=32) as wpool,
            tc.tile_pool(name="xpool", bufs=4) as xpool,
            tc.tile_pool(name="bpool", bufs=1) as bpool,
            tc.tile_pool(name="opool", bufs=3) as opool,
            tc.tile_pool(name="psum", bufs=8, space="PSUM") as psum_pool,
        ):
            # Tiny warmup DMA on each queue: absorbs cold DGE/queue init
            # on throwaway transfers instead of the critical first W/x.
            for i, eng in enumerate((nc.sync, nc.scalar, nc.gpsimd)):
                wu = bpool.tile([P, 8], F32, tag=f"wu{i}", name=f"wu{i}")
                eng.dma_start(wu[:], bias.ap()[:, ts(i, 8)])

            # PE p-state warmup: ~10 dummy matmuls on a zeroed scratch
            # tile keep the PE active while the real W/x DMAs are in
            # flight, so the first real matmuls run at full clock.
            scr = bpool.tile([P, P], BF16, tag="scr", name="scr")
            nc.gpsimd.memset(scr[:], 0)
            dummy_acc = psum_pool.tile([P, 512], F32, tag="acc", name="acc")
            for _ in range(10):
                nc.tensor.matmul(
                    dummy_acc[:, :P], scr[:], scr[:], start=True, stop=True
                )

            # First x tile in 4 chunks ahead of everything else on the
            # sync HWDGE queue (~0.6us first-byte vs ~2us SWDGE, and
            # measured 2x faster through the cold DMA-engine ramp) so
            # the PE's first stationary operand arrives within ~10us.
            x_first = xpool.tile([P, KT, P], BF16, tag="x", name="x_first")
            for c in range(4):
                nc.sync.dma_start(
                    x_first[:, ts(c, KT // 4), :],
                    xt.ap()[0, :, ts(c, KT // 4), :],
                )

            bias_sb = bpool.tile([P, DOUT], F32)

            # ---- W piece DMAs -> per-(phase,k) moving-operand slices
            w_slice = {}           # (ph, k) -> (tile, elem offset)
            piece_by_phase = [[] for _ in PHASES]
            for pi, (ph, k0, g) in enumerate(PIECES):
                piece_by_phase[ph].append((pi, k0, g))

            def issue_piece(ph, idx, eng):
                olen = PHASES[ph][1]
                pi, k0, g = piece_by_phase[ph][idx]
                wtile = wpool.tile(
                    [P, g * olen], BF16, tag="w", name="w",
                    padded_shape=[P, 2048],
                )
                eng.dma_start(
                    wtile[:],
                    wp.ap()[:, PIECE_OFF[pi] : PIECE_OFF[pi] + g * olen],
                )
                for j in range(g):
                    w_slice[(ph, k0 + j)] = (wtile, j * olen)

            def issue_phase_w(ph, engines, last_on_gpsimd=False):
                n = len(piece_by_phase[ph])
                for idx in range(n):
                    if last_on_gpsimd and idx == n - 1:
                        eng = nc.gpsimd
                    else:
                        eng = engines[idx % len(engines)]
                    issue_piece(ph, idx, eng)

            for ph, (o0, olen) in enumerate(PHASES):
                if ph == 0:
                    # Phase-0 W, need-ordered: singles k0-5 on scalar
                    # (sync's first lanes carry the x chunks), groups
                    # interleaved sync/scalar, tail group on SWDGE.
                    ph0_eng = [nc.scalar] * 6 + [
                        nc.sync, nc.scalar, nc.sync, nc.scalar,
                        nc.sync, nc.gpsimd, nc.gpsimd,
                    ]
                    for idx, eng in enumerate(ph0_eng):
                        issue_piece(0, idx, eng)
                    # first bias half behind phase-0's W on scalar
                    # (lands well before the first eviction needs it)
                    nc.scalar.dma_start(
                        bias_sb[:, 0:1024], bias.ap()[:, 0:1024]
                    )
                # Phases >= 1 are issued near the end of t==0 of the
                # previous phase (below): by then their pool slots are
                # provably free (no WAR wait can block the HWDGE FIFO)
                # and the stream still has nearly a full phase of
                # compute (~95-200us) to land before first use.

                banks = [512] * (olen // 512)
                for t in range(NT):
                    if ph == 0 and t == 0:
                        xtile = x_first
                    else:
                        # alternate the x stream across SWDGE and the
                        # sync HWDGE ring so W prefetch bursts on one
                        # queue cannot starve the stationary operand
                        xtile = xpool.tile([P, KT, P], BF16, tag="x")
                        xeng = nc.sync if t % 2 == 1 else nc.gpsimd
                        xeng.dma_start(xtile[:], xt.ap()[t])

                    accs = [
                        psum_pool.tile([P, 512], F32, tag="acc", name="acc")
                        for _ in banks
                    ]
                    for k in range(KT):
                        wtile, eoff = w_slice[(ph, k)]
                        for b, blen in enumerate(banks):
                            nc.tensor.matmul(
                                accs[b][:],
                                xtile[:, k, :],                      # stationary
                                wtile[:, eoff + b * blen : eoff + (b + 1) * blen],
                                start=(k == 0),
                                stop=(k == KT - 1),
                            )

                    last = ph == len(PHASES) - 1 and t == NT - 1
                    if last:
                        # split the final eviction across both HWDGE
                        # queues to shrink the drain tail
                        for b, blen in enumerate(banks):
                            osb = opool.tile(
                                [P, blen], F32, tag="o", name="o",
                                padded_shape=[P, 1024],
                            )
                            nc.vector.tensor_add(
                                osb[:],
                                accs[b][:],
                                bias_sb[:, o0 + b * blen : o0 + (b + 1) * blen],
                            )
                            eng = nc.sync if b % 2 == 0 else nc.scalar
                            eng.dma_start(
                                out.ap()[
                                    ts(t, P), o0 + b * blen : o0 + (b + 1) * blen
                                ],
                                osb[:],
                            )
                    else:
                        osb = opool.tile(
                            [P, olen], F32, tag="o", name="o",
                            padded_shape=[P, 1024],
                        )
                        for b, blen in enumerate(banks):
                            nc.vector.tensor_add(
                                osb[:, ts(b, blen)],
                                accs[b][:],
                                bias_sb[:, o0 + b * blen : o0 + (b + 1) * blen],
                            )
                        # out rides scalar only: eviction-paced, so it
                        # also throttles the W prefetch pieces queued
                        # behind it (sync's FIFO stays clear for x)
                        nc.scalar.dma_start(
                            out.ap()[ts(t, P), o0 : o0 + olen], osb[:]
                        )

                    if ph + 1 < len(PHASES) and t < len(
                        piece_by_phase[ph + 1]
                    ):
                        # Prefetch the next phase's W, PACED at one
                        # ~512KB piece per t-tile: a bulk prefetch
                        # burst monopolizes the shared SDMA engines
                        # and starves the x stream. With 32 pool
                        # slots every prefetch reuses a slot freed
                        # two phases ago, so no WAR wait can stall
                        # the HWDGE FIFO behind this.
                        eng = nc.sync if t % 2 == 0 else nc.scalar
                        issue_piece(ph + 1, t, eng)
                    if ph == 0 and t == 1:
                        # rest of the bias, first needed in phase 2
                        nc.scalar.dma_start(
                            bias_sb[:, 1024:DOUT],
                            bias.ap()[:, 1024:DOUT],
                        )

    nc.compile()
    return nc


def _pack_weight(weight):
    wt = weight.T.astype(ml_dtypes.bfloat16)           # [DIN, DOUT]
    wk = wt.reshape(KT, P, DOUT)                       # [k, p, o]
    wpack = np.empty((P, WPACK_F), dtype=ml_dtypes.bfloat16)
    for pi, (ph, k0, g) in enumerate(PIECES):
        o0, olen = PHASES[ph]
        blk = wk[k0 : k0 + g, :, o0 : o0 + olen]       # [g, p, olen]
        wpack[:, PIECE_OFF[pi] : PIECE_OFF[pi] + g * olen] = (
            blk.transpose(1, 0, 2).reshape(P, g * olen)
        )
    return np.ascontiguousarray(wpack)


def kernel(x, weight, bias):
    global _NC, LAST_RESULT
    if _NC is None:
        _NC = _build_nc()

    X = np.ascontiguousarray(x.reshape(B * S, DIN))
    wpack = _pack_weight(weight)
    bias_rep = np.ascontiguousarray(
        np.broadcast_to(bias.astype(np.float32), (P, DOUT))
    )
    in_maps = []
    for c in range(NCORES):
        xc = X[c * T : (c + 1) * T].astype(ml_dtypes.bfloat16)
        # [t-tile, p(=k%128), ks, i(=token%128)]
        xt_c = np.ascontiguousarray(
            xc.reshape(NT, P, KT, P).transpose(0, 3, 2, 1)
        )
        in_maps.append({"xt": xt_c, "wpack": wpack, "bias_rep": bias_rep})

    last_err = None
    for _attempt in range(2):
        try:
            res = run_bass_kernel_spmd(_NC, in_maps, list(range(NCORES)))
            break
        except Exception as e:  # transient NRT device errors: retry once
            last_err = e
    else:
        raise last_err
    LAST_RESULT = res

    out = np.concatenate([res.results[c]["out"] for c in range(NCORES)], axis=0)
    return out.reshape(B, S, DOUT).astype(np.float32, copy=False)
